# revision 10
# baseline (speedup 1.0000x reference)
"""Trainium2 Bass kernel for nn_DensityVQC (batched 2-qubit VQC Z-expectation).

Algebra
-------
The reference builds rho_b = conj(psi_b) psi_b^T (note: transpose of the
standard density matrix), evolves rho' = U rho U^dag and returns
tr(rho' Z0) with Z0 = diag(1,1,-1,-1).  This collapses to a per-row
quadratic form: with V = conj(U) (the transposed-rho convention flips the
conjugation) and phi = V psi,

    out_b = |phi_0|^2 + |phi_1|^2 - |phi_2|^2 - |phi_3|^2
          = 2 * || C psi_b ||^2 - ||psi_b||^2        (C = V[0:2, :], U unitary)
          = || A r_b + B m_b ||^2 - 1                (inputs are unit-norm)

with real 4x4 matrices A = sqrt(2)*[Re C; Im C], B = sqrt(2)*[-Im C; Re C].
So the device kernel is: per batch row (r, m in R^4), compute w = A r + B m,
then out = sum(w^2) - 1.  No [B,4,4] density matrices are ever materialized.

Device mapping (per core, pure data parallel over 8 cores)
----------------------------------------------------------
Everything on the wire and through the PE runs in fp16 (measured end-to-end
rel err ~1e-3, tolerance 2e-2): halves HBM traffic vs f32 AND runs matmuls
at 1 col/cycle (fp32 "HIGH" mode costs 2-4x on HW).

Per-queue DMA bandwidth scales with the per-partition descriptor size
(2KB segments -> ~105 GB/s, 4KB -> ~165 GB/s), and a big transfer kicked
first can hog the DMA-engine pool, so rt and mt stream as separate
tensors on the sync and gpsimd queues with small leading chunks (early
compute start) and wide 4KB-descriptor middle chunks.  The consts ride a
third (scalar-queue) DMA off the data path.  out_lo stores via gpsimd;
out_hi (the critical tail store) via the scalar queue.

The PE p-state demotes to 1.2 GHz on any issue gap, so besides the
start-up warm-up burst, small filler matmuls are interleaved between
pairs to hold the clock at 2.4 GHz across chunk-arrival gaps.

Per supertile pair (2 x 512 free columns = 32768 batch rows):
  1. PE: 4 accumulating fp16 matmuls into a [128,1024] 2-bank PSUM tile
  2. ONE ACT Square [128,1024] -> fp16 SBUF (squares are ACT-only: DVE
     cannot read two PSUM operands); the last pair squares in two 512-col
     halves to shorten the tail
  3. PE: two reduce matmuls (stationary = signed group-sum pattern, fp16);
     4 supertiles accumulate into one full-width [128,512] PSUM bank
  4. copy with -1 fold -> [128,512] fp16 output tile (the final copy is
     split between Scalar and Vector), DMA out on the vector queue
A burst of tiny warm-up matmuls at kernel start ramps the PE clock
(0.65 -> 2.4 GHz takes ~3 us of sustained issue) before real data lands.
The host un-permutes the fp16 output back to batch order and upcasts.
"""

import sys
import numpy as np

if "/opt/trn_rl_repo" not in sys.path:
    sys.path.insert(0, "/opt/trn_rl_repo")

import concourse.bass as bass
import concourse.tile as tile
from concourse import bacc, mybir
from concourse import bass_utils
from concourse.tile_rust import add_dep_helper

N_CORES = 8
BSZ = 1_048_576
BC = BSZ // N_CORES            # 131072 rows per core
NCOL = BC // 32                # 4096 component-major free columns per tensor
N_PAIR = 4                     # supertile pairs; pair = 2048 xt columns
CBASE = 768                    # const columns: zq(512) + ablk(128) + bblk(128)
XCOLS = CBASE + 2 * NCOL       # 8960
N_WARM = 8
F32 = mybir.dt.float32
F16 = mybir.dt.float16
N_LAYERS = 6


def _circuit_unitary(ry, rz):
    """4x4 circuit unitary, float64 mirror of reference._circuit_unitary."""
    ry = np.asarray(ry, dtype=np.float64)
    rz = np.asarray(rz, dtype=np.float64)
    cnot = np.array(
        [[1, 0, 0, 0], [0, 1, 0, 0], [0, 0, 0, 1], [0, 0, 1, 0]],
        dtype=np.complex128,
    )

    def _ry(th):
        c, s = np.cos(th / 2), np.sin(th / 2)
        return np.array([[c, -s], [s, c]], dtype=np.complex128)

    def _rz(th):
        return np.diag([np.exp(-0.5j * th), np.exp(0.5j * th)])

    u = np.eye(4, dtype=np.complex128)
    for l in range(ry.shape[0]):
        ry_full = np.kron(_ry(ry[l, 0]), _ry(ry[l, 1]))
        rz_full = np.kron(_rz(rz[l, 0]), _rz(rz[l, 1]))
        u = cnot @ (rz_full @ (ry_full @ u))
    return u


def _host_consts(ry_params, rz_params):
    u = _circuit_unitary(ry_params, rz_params)
    c = np.conj(u)[0:2, :]
    a = np.sqrt(2.0) * np.vstack([c.real, c.imag])     # 4x4, w = A r + B m
    b = np.sqrt(2.0) * np.vstack([-c.imag, c.real])
    eye32 = np.eye(32, dtype=np.float32)
    # lhsT[k=4g+c, m=4g+j] = A[j, c]  ->  block_diag of A.T
    ablk = np.kron(eye32, a.T).astype(np.float16)
    bblk = np.kron(eye32, b.T).astype(np.float16)
    zsum = np.kron(eye32, np.ones((4, 1), dtype=np.float32))
    # Four partition-shifted reduce patterns: zq[k, 32q+g] = zsum[k, g].
    # Supertile st (q = st%4) accumulates its group-sums into partitions
    # [32q, 32q+32) of a shared full-width PSUM bank.
    zqs = []
    for q in range(4):
        zq = np.zeros((128, 128), dtype=np.float32)
        zq[:, 32 * q : 32 * (q + 1)] = zsum
        zqs.append(zq.astype(np.float16))
    return ablk, bblk, zqs


# Any fixed permutation of the 4096 32-row blocks works (the host inverts
# it); identity keeps the input marshalling a pure reshape+transpose.
def _to_component_major(x):
    """x [BC,4] f32 -> [128, NCOL] fp16: column N holds batch rows
    [32N, 32N+32) x 4 comps on the 128 partitions."""
    return np.ascontiguousarray(x.reshape(NCOL, 128).T.astype(np.float16))


def _from_out32(y):
    """y [2, 128, 512] -> [BC]: value for supertile st = 4h+q, col n, group g
    lives at y[h, 32q+g, n]; batch b = 16384*st + 32n + g."""
    return np.ascontiguousarray(
        y.astype(np.float32).reshape(2, 4, 32, 512).transpose(0, 1, 3, 2)
    ).reshape(-1)


def _build_program():
    nc = bacc.Bacc("TRN2", target_bir_lowering=False, debug=False)
    # zq+ablk+bblk consts live in their own tensor on the scalar queue.
    ct_d = nc.dram_tensor("ct", [128, CBASE], F16, kind="ExternalInput")
    rt_d = nc.dram_tensor("rt", [128, NCOL], F16, kind="ExternalInput")
    mt_d = nc.dram_tensor("mt", [128, NCOL], F16, kind="ExternalInput")
    out_d = nc.dram_tensor("out", [2, 128, 512], F16, kind="ExternalOutput")

    out_lo_d = out_d.ap()[0]
    out_hi_d = out_d.ap()[1]

    with tile.TileContext(nc) as tc:
        with (
            tc.tile_pool(name="io", bufs=1) as iopool,
            tc.tile_pool(name="work", bufs=4) as wpool,
            tc.tile_pool(name="psum", bufs=2, space=bass.MemorySpace.PSUM) as ppool,
        ):
            ct_t = iopool.tile([128, CBASE], F16, name="ct_t")
            rt_t = iopool.tile([128, NCOL], F16, name="rt_t")
            mt_t = iopool.tile([128, NCOL], F16, name="mt_t")
            zq = [ct_t[:, 128 * q : 128 * (q + 1)] for q in range(4)]
            ablk = ct_t[:, 512:640]
            bblk = ct_t[:, 640:768]
            out_lo = iopool.tile([128, 512], F16, name="out_lo")
            out_hi = iopool.tile([128, 512], F16, name="out_hi")

            # PE clock warm-up: the p-state ramps only under sustained
            # issue, so burn 512-col matmuls on a memset tile across the
            # first-chunk load window.
            warm = wpool.tile([128, 512], F16, name="warm", bufs=1)
            nc.gpsimd.memset(warm[:], 1.0)
            wps = [
                ppool.tile([128, 512], F32, name=f"warm_ps{i}", bufs=1)
                for i in range(2)
            ]
            for w in range(N_WARM):
                nc.tensor.matmul(wps[w % 2][:], warm[:, 0:128], warm[:])

            nc.scalar.dma_start(ct_t[:], ct_d.ap()[:])
            # Small first chunks (supertile 0) for an early compute start,
            # then wide 4KB-descriptor chunks.
            cb = [0, 512, 2560, 4096]
            prev_r, prev_m = None, None
            for c in range(len(cb) - 1):
                cs = bass.ds(cb[c], cb[c + 1] - cb[c])
                r_dma = nc.sync.dma_start(rt_t[:, cs], rt_d.ap()[:, cs])
                m_dma = nc.gpsimd.dma_start(mt_t[:, cs], mt_d.ap()[:, cs])
                if prev_r is not None:
                    add_dep_helper(r_dma.ins, prev_r.ins, sync=False, reason="q")
                    add_dep_helper(m_dma.ins, prev_m.ins, sync=False, reason="q")
                prev_r, prev_m = r_dma, m_dma

            def rt_sl(p, h):
                return rt_t[:, bass.ds(512 * (2 * p + h), 512)]

            def mt_sl(p, h):
                return mt_t[:, bass.ds(512 * (2 * p + h), 512)]

            for p in range(N_PAIR):
                phi = ppool.tile([128, 1024], F32, name="phi", bufs=2)
                for h in range(2):
                    hs = bass.ds(512 * h, 512)
                    nc.tensor.matmul(
                        phi[:, hs], ablk, rt_sl(p, h), start=True, stop=False
                    )
                    nc.tensor.matmul(
                        phi[:, hs], bblk, mt_sl(p, h), start=False, stop=True
                    )

                s_sb = wpool.tile([128, 1024], F16, name="s_sb", bufs=3)
                if p < N_PAIR - 1:
                    nc.scalar.activation(
                        s_sb[:], phi[:], mybir.ActivationFunctionType.Square
                    )
                else:
                    # Last pair: square in halves so the final reduce/copy
                    # chain starts as early as possible.
                    for h in range(2):
                        hs = bass.ds(512 * h, 512)
                        nc.scalar.activation(
                            s_sb[:, hs], phi[:, hs],
                            mybir.ActivationFunctionType.Square,
                        )

                if p % 2 == 0:
                    ored = ppool.tile([128, 512], F32, name="ored", bufs=2)
                for h in range(2):
                    q = (2 * p + h) % 4
                    nc.tensor.matmul(
                        ored[:], zq[q], s_sb[:, bass.ds(512 * h, 512)],
                        start=(q == 0), stop=(q == 3),
                    )

                if p < N_PAIR - 1:
                    # Clock-hold fillers bridging the next chunk-arrival gap.
                    for w in range(3):
                        nc.tensor.matmul(wps[w % 2][:], warm[:, 0:128], warm[:])

                if p == 1:
                    nc.vector.tensor_scalar_add(out_lo[:], ored[:], -1.0)
                    nc.gpsimd.dma_start(out_lo_d, out_lo[:])
                if p == 3:
                    # Final copy split across both engines, then store.
                    nc.scalar.activation(
                        out_hi[:, 0:256],
                        ored[:, 0:256],
                        mybir.ActivationFunctionType.Copy,
                        bias=-1.0,
                    )
                    nc.vector.tensor_scalar_add(
                        out_hi[:, 256:512], ored[:, 256:512], -1.0
                    )
                    nc.scalar.dma_start(out_hi_d, out_hi[:])
    nc.compile()
    return nc


_PROG_CACHE = None


def _get_program():
    global _PROG_CACHE
    if _PROG_CACHE is None:
        _PROG_CACHE = _build_program()
    return _PROG_CACHE


def _run(ry_params, rz_params, states_real, states_imag, **hw_kwargs):
    ablk, bblk, zqs = _host_consts(ry_params, rz_params)
    ct = np.concatenate(zqs + [ablk, bblk], axis=1).astype(np.float16)
    states_real = np.ascontiguousarray(states_real, dtype=np.float32)
    states_imag = np.ascontiguousarray(states_imag, dtype=np.float32)
    in_maps = []
    for k in range(N_CORES):
        sl = slice(k * BC, (k + 1) * BC)
        in_maps.append(
            {
                "ct": ct,
                "rt": _to_component_major(states_real[sl]),
                "mt": _to_component_major(states_imag[sl]),
            }
        )
    nc = _get_program()
    res = bass_utils.run_bass_kernel_spmd(
        nc, in_maps, core_ids=list(range(N_CORES)), **hw_kwargs
    )
    out = np.concatenate(
        [_from_out32(res.results[k]["out"]) for k in range(N_CORES)]
    ).astype(np.float32)
    return out, res


def kernel(ry_params, rz_params, states_real, states_imag):
    out, _ = _run(ry_params, rz_params, states_real, states_imag)
    return out
